# revision 10
# baseline (speedup 1.0000x reference)
"""Multi-head self-attention (B=2, S=4096, D=512, H=8, Dh=64) on 8 TRN2 cores.

Sharding: core i handles batch b = i//4 and head-pair hp = i%4 (heads 2*hp,
2*hp+1).  Each core computes Q/K/V projections for its two heads, flash-style
attention (no-max softmax; scores range is +-9 so exp is safe), and a partial
out-projection.  Host sums the 4 partial outputs per batch and transposes back.

v2 (bf16 pipeline): all matmul operands are bfloat16.  bf16 matmuls use
separate LDWEIGHTS instructions that the PE's 64-deep reorder window pulls
into the background weight buffer (plus FWL), so weight loads hide under the
streaming passes -- unlike fp32r matmuls, which self-load weights serially
(~107ns per matmul).  The kernel is restructured so the ACT engine (exp,
1 elem/cycle/lane @1.2GHz = the softmax roofline, ~290us for 33.5M exps)
never stalls:
  - scores PSUM tiles double-buffered, ctx PSUM double-buffered (normalize of
    block i runs under block i+1's compute; no PE idle -> no HAM re-throttle)
  - K projection accumulates chunk-by-chunk as the X DMA lands
  - out-projection of q-block i runs on the spare ctx-ring slot inside block
    i+1, copies on DVE (never ACT), output DMA overlapped
  - softmax normalize via reciprocal_approx_fast + DRAM-bounce partition
    broadcast, all off the critical path

Layouts (feature dim on partitions; every matmul contracts on partitions):
  xt  [512, S]  = X[b].T                       (bf16)
  wq/wk/wv [512, 128] = W[:, hp*128:(hp+1)*128] (bf16)
  wo  [128, 512] = Wo[hp*128:(hp+1)*128, :]     (bf16)
  yt  [512, S]  = partial (Y[b]).T              (fp32)

TRN2 quirk: walrus legalizes only ONE sync wait on TPB compute instructions.
`_legalize_matmul_waits` moves extra waits onto injected single-wait no-ops.
"""

import sys
from contextlib import ExitStack

for _p in ("/opt/trn_rl_repo",):
    if _p not in sys.path:
        sys.path.insert(0, _p)

import numpy as np

import concourse.bass as bass
import concourse.tile as tile
from concourse import mybir
from concourse.bass_utils import run_bass_kernel_spmd

F32 = mybir.dt.float32
BF16 = mybir.dt.bfloat16
MM_DT = BF16
D = 512          # model dim
DH = 64          # head dim
P = 128          # partitions
B = 2
H = 8
S_FULL = 4096
N_CORES = 8
NC_T = D // P    # 4 contraction tiles over model dim

LAST_RESULTS = None  # test harness reads exec_time_ns from here


def _emit(nc: bass.Bass, tc: "tile.TileContext", ctx: ExitStack, S: int):
    """Emit the per-core program. Parameterized by S for small-sim testing."""
    NK = S // P              # 128-row key tiles
    QB = 512                 # q-block (both heads processed per block)
    NQB = S // QB            # attention q-blocks
    inv_scale = 1.0 / np.sqrt(DH)

    def mm(out, lhsT, rhs, start=True, stop=True):
        return nc.tensor.matmul(out, lhsT, rhs, start=start, stop=stop)

    xt = nc.declare_dram_parameter("xt", [D, S], MM_DT, isOutput=False)
    wq = nc.declare_dram_parameter("wq", [D, P], MM_DT, isOutput=False)
    wk = nc.declare_dram_parameter("wk", [D, P], MM_DT, isOutput=False)
    wv = nc.declare_dram_parameter("wv", [D, P], MM_DT, isOutput=False)
    wo = nc.declare_dram_parameter("wo", [P, D], MM_DT, isOutput=False)
    yt = nc.declare_dram_parameter("yt", [D, S], F32, isOutput=True)

    const = ctx.enter_context(tc.tile_pool(name="const", bufs=1))

    # ---- weight DMA first (small), then X in [128,1024] pieces so the
    # projections that need only the first column-block can start early ----
    w_sb = {}
    for name, ap in (("wq", wq), ("wk", wk), ("wv", wv)):
        tiles = []
        for c in range(NC_T):
            t = const.tile([P, P], MM_DT, tag=f"{name}{c}", name=f"{name}{c}")
            nc.sync.dma_start(out=t[:], in_=ap[c * P:(c + 1) * P, :])
            tiles.append(t)
        w_sb[name] = tiles
    wo_sb = const.tile([P, D], MM_DT, tag="wo")
    nc.sync.dma_start(out=wo_sb[:], in_=wo[:, :])
    xt_sb = [const.tile([P, S], MM_DT, tag=f"xt{c}", name=f"xt{c}")
             for c in range(NC_T)]
    XPIECE = min(1024, S)
    for b in range(S // XPIECE):
        for c in range(NC_T):
            sl = slice(b * XPIECE, (b + 1) * XPIECE)
            nc.sync.dma_start(out=xt_sb[c][:, sl], in_=xt[c * P:(c + 1) * P, sl])

    # persistent intermediates
    qt_sb = const.tile([P, S], MM_DT, tag="qt")      # [2*64 d, S] stacked heads
    kt_sb = const.tile([P, S], MM_DT, tag="kt")
    # V with a ones column appended per k-tile: [128 k, NK*65]; col 64 == 1.0
    vones = [const.tile([P, NK * (DH + 1)], MM_DT, tag=f"vones{h}", name=f"vones{h}")
             for h in range(2)]
    konst = const.tile([P, NK, 1], F32, tag="konst")
    nc.vector.memset(konst[:], 1.0)
    for h in range(2):
        vv = vones[h].rearrange("p (k c) -> p k c", c=DH + 1)
        nc.vector.tensor_copy(vv[:, :, DH:DH + 1], konst[:])
    ctx_sb = const.tile([P, S], MM_DT, tag="ctx")    # context^T, stacked heads

    # PSUM banks: "s" 2 x [128,1024] (4) + "ctx" 3 x [65,512] (3) + "pp" 1 x
    # [128,512] (1) = 8.  Buffer *addresses* are assigned by autobufs
    # (interval packing); bufs= here are the per-tag caps.
    ps = ctx.enter_context(tc.tile_pool(name="ps", bufs=2, space="PSUM"))
    es = ctx.enter_context(tc.tile_pool(name="es", bufs=8))
    bcp = ctx.enter_context(tc.tile_pool(name="bcp", bufs=2))
    rtp = ctx.enter_context(tc.tile_pool(name="rtp", bufs=2))
    rdp = ctx.enter_context(tc.tile_pool(name="rdp", bufs=2, space="DRAM"))
    osb = ctx.enter_context(tc.tile_pool(name="osb", bufs=4))

    # ---- projections (tag "pp", 1 bank; they fill PE slack under the
    # ACT-bound attention loop, racing ahead of their consumers) ----
    def proj_block(dst, wname, lo):
        """dst[:, lo:lo+512] = (W^T x)[:, lo:lo+512] over the 4 chunks."""
        pq = ps.tile([P, 512], F32, tag="pp", bufs=1, name=f"p{wname}")
        for c in range(NC_T):
            mm(pq[:], w_sb[wname][c][:], xt_sb[c][:, lo:lo + 512],
               start=(c == 0), stop=(c == NC_T - 1))
        nc.vector.tensor_copy(dst[:, lo:lo + 512], pq[:])

    def proj_v(k):
        ksl = slice(k * P, (k + 1) * P)
        pv = ps.tile([P, P], F32, tag="pp", bufs=1, name="pv")
        for c in range(NC_T):
            mm(pv[:], xt_sb[c][:, ksl], w_sb["wv"][c][:],
               start=(c == 0), stop=(c == NC_T - 1))
        for h in range(2):
            nc.vector.tensor_copy(
                vones[h][:, k * (DH + 1):k * (DH + 1) + DH],
                pv[:, h * DH:(h + 1) * DH])

    # minimal upfront set: K/Q for the first q-block, first two V k-tiles
    NSB = S // 512
    proj_block(kt_sb, "wk", 0)
    if NSB > 1:
        proj_block(kt_sb, "wk", 512)
    proj_v(0)
    proj_v(1)
    proj_block(qt_sb, "wq", 0)
    # the rest, in consumption order (K block bp feeds score k-tiles 8bp+)
    for k in range(2, min(8, NK)):
        proj_v(k)
    for bp in range(2, NSB):
        proj_block(kt_sb, "wk", bp * 512)
        for k in range(4 * bp, 4 * bp + 4):
            proj_v(k)
    for bp in range(1, NSB):
        proj_block(qt_sb, "wq", bp * 512)

    # ---- phase B + C interleaved ----
    # Per (qb, k): the two heads' score matmuls are row-packed -- h0 uses PE
    # rows 0-63 (base_partition 0), h1 rows 64-127 (base_partition 64) -- and
    # run CONCURRENTLY in disjoint row-groups, writing the two 512-col halves
    # (= two different banks) of one [128,1024] PSUM tile.  A single N=1024
    # exp then covers both heads, keeping the ACT cadence at (1024+352)/1.2
    # ~= 1147ns per k-tile while PE streaming is only ~650ns (fits under the
    # ACT cadence even when the power manager halves the PE clock).
    def emit_out_tile(qb, idx):
        """One out-projection column tile for q-block qb on the "s" ring."""
        sl = slice(qb * QB, (qb + 1) * QB)
        o_ps = ps.tile([P, QB], F32, tag="s", name="o_ps")
        mm(o_ps[:, :QB], wo_sb[:, idx * P:(idx + 1) * P], ctx_sb[:, sl])
        o_sb = osb.tile([P, QB], F32, tag="osb", name="o_sb")
        nc.vector.tensor_copy(o_sb[:], o_ps[:, :QB])
        nc.sync.dma_start(out=yt[idx * P:(idx + 1) * P, sl], in_=o_sb[:])

    def normalize(h, qb, ctx_ps):
        """Emit rowsum reciprocal + partition-broadcast for one head-block.
        The [1,QB] rowsum row is reshaped to [64, QB//64] via a DRAM bounce
        so the DVE reciprocal costs ~QB//64 columns, not QB.  Returns the
        final tensor_mul emission (deferred by the caller to keep the DVE
        FIFO clear of long waits)."""
        hsl = slice(h * DH, (h + 1) * DH)
        qsl = slice(qb * QB, (qb + 1) * QB)
        NW = QB // DH        # columns per partition in the [64, NW] reshape
        rt = rtp.tile([1, QB], F32, tag="rt", name="rt")
        nc.vector.tensor_copy(rt[0:1, :], ctx_ps[DH:DH + 1, :])
        rd1 = rdp.tile([1, QB], F32, tag="rd1", name="rd1")
        nc.sync.dma_start(out=rd1[:], in_=rt[0:1, :])
        el = list(rd1[0:1, :].ap)[1]           # [elem_stride, QB]
        rd1_64 = bass.AP(tensor=rd1.tensor, offset=rd1.offset,
                         ap=[[el[0] * NW, DH], [el[0], NW]])
        r64 = rtp.tile([DH, NW], F32, tag="r64", name="r64")
        nc.sync.dma_start(out=r64[:], in_=rd1_64)
        rinv64 = rtp.tile([DH, NW], F32, tag="rinv64", name="rinv64")
        nc.vector.reciprocal(rinv64[:], r64[:])
        rd2 = rdp.tile([1, QB], F32, tag="rd2", name="rd2")
        el2 = list(rd2[0:1, :].ap)[1]
        rd2_64 = bass.AP(tensor=rd2.tensor, offset=rd2.offset,
                         ap=[[el2[0] * NW, DH], [el2[0], NW]])
        nc.sync.dma_start(out=rd2_64, in_=rinv64[:])
        rd2_bcast = bass.AP(tensor=rd2.tensor, offset=rd2.offset,
                            ap=[[0, DH], el2])
        bc = bcp.tile([DH, QB], F32, tag="bc", name="bc")
        nc.sync.dma_start(out=bc[:], in_=rd2_bcast)
        return lambda: nc.vector.tensor_mul(ctx_sb[hsl, qsl],
                                            ctx_ps[:DH, :], bc[:])

    # Scheduler-time estimates for tile_wait_until placement hints (the tile
    # scheduler reorders by its own simulated timeline; its DMA model is
    # optimistic, so without a hint it parks the out-projection matmuls --
    # which wait on the normalize muls -- at the head of the next block's PE
    # queue, stalling the exp stream ~4-5us per boundary).
    HEAD_US = 12.0
    BLOCK_US = NK * 1.15
    for qb in range(NQB):
        qsl = slice(qb * QB, (qb + 1) * QB)
        ctx_h = [ps.tile([DH + 1, QB], F32, tag="ctx", bufs=3,
                         name=f"ctx_ps{h}") for h in range(2)]
        for k in range(NK):
            s_pair = ps.tile([P, 2 * QB], F32, tag="s", name="s_pair")
            for h in range(2):
                hsl = slice(h * DH, (h + 1) * DH)
                mm(s_pair[:, h * QB:(h + 1) * QB],
                   kt_sb[hsl, k * P:(k + 1) * P], qt_sb[hsl, qsl])
            e_pair = es.tile([P, 2 * QB], MM_DT, tag="e", name="e_pair")
            nc.scalar.activation(e_pair[:], s_pair[:],
                                 mybir.ActivationFunctionType.Exp,
                                 scale=inv_scale)
            for h in range(2):
                vo = vones[h][:, k * (DH + 1):(k + 1) * (DH + 1)]
                mm(ctx_h[h][:], vo, e_pair[:, h * QB:(h + 1) * QB],
                   start=(k == 0), stop=(k == NK - 1))
        # normalize both heads: long-latency DMA chains first, muls last so
        # the in-order DVE queue never parks on a DMA wait ahead of the
        # out-projection copies
        muls = [normalize(h, qb, ctx_h[h]) for h in range(2)]
        for m in muls:
            m()
        # this q-block's out-projection, placed ~1/4 into the next block's
        # timeline (its ctx_sb inputs arrive ~5us after the boundary)
        if qb < NQB - 1:
            wait_ms = (HEAD_US + BLOCK_US * (qb + 1) + 0.3 * BLOCK_US) / 1000
            with tc.tile_wait_until(wait_ms):
                for idx in range(NC_T):
                    emit_out_tile(qb, idx)
        else:
            for idx in range(NC_T):
                emit_out_tile(qb, idx)


_TPB_ENGINES = {mybir.EngineType.PE, mybir.EngineType.Activation,
                mybir.EngineType.DVE, mybir.EngineType.Pool}


def _legalize_matmul_waits(nc: bass.Bass) -> int:
    """Walrus encodes only ONE sync wait on TPB compute instructions (seen on
    Matmult and TensorCopy).  Move extra waits onto injected same-engine
    no-ops (one wait each) placed immediately before the instruction in its
    block: same semantics, legal encoding."""
    n_fixed = 0
    for f in nc.m.functions:
        for bb in f.blocks:
            out = []
            changed = False
            for ins in bb.instructions:
                si = ins.sync_info
                if (getattr(ins, "engine", None) is not None
                        and si is not None and len(si.on_wait) > 1):
                    for idx, w in enumerate(si.on_wait[:-1]):
                        nop = mybir.InstNoOp(name=f"{ins.name}-lgw{idx}",
                                             ins=[], outs=[])
                        nop.engine = ins.engine
                        nop.sync_info = mybir.SyncInfo(on_wait=[w], on_update=[])
                        out.append(nop)
                    ins.sync_info = mybir.SyncInfo(on_wait=[si.on_wait[-1]],
                                                   on_update=si.on_update)
                    n_fixed += 1
                    changed = True
                out.append(ins)
            if changed:
                bb.instructions = out
    return n_fixed


def build(S: int = S_FULL, legalize: bool = False) -> bass.Bass:
    nc = bass.Bass()
    with ExitStack() as ctx:
        ctx.enter_context(nc.allow_low_precision(
            reason="bf16 matmul operands / intermediates"))
        tc = ctx.enter_context(tile.TileContext(nc))
        _emit(nc, tc, ctx, S)
    if legalize:
        # only for the walrus/hardware path; CoreSim wants updates on every
        # instruction and doesn't enforce the 1-wait Matmult limit
        _legalize_matmul_waits(nc)
    return nc


_NC_CACHE = {}


def _get_nc(S: int) -> bass.Bass:
    if S not in _NC_CACHE:
        _NC_CACHE[S] = build(S, legalize=True)
    return _NC_CACHE[S]


def _bf16(a):
    import ml_dtypes
    return np.ascontiguousarray(np.asarray(a, dtype=np.float32)).astype(
        ml_dtypes.bfloat16)


def make_in_maps(X, Wq, Wk, Wv, Wo):
    X = np.asarray(X, dtype=np.float32)
    xts = [_bf16(X[b].T) for b in range(B)]
    in_maps = []
    for i in range(N_CORES):
        b, hp = divmod(i, 4)  # 4 head-pairs per batch
        csl = slice(hp * P, (hp + 1) * P)
        in_maps.append({
            "xt": xts[b],
            "wq": _bf16(Wq[:, csl]),
            "wk": _bf16(Wk[:, csl]),
            "wv": _bf16(Wv[:, csl]),
            "wo": _bf16(Wo[csl, :]),
        })
    return in_maps


def kernel(X, Wq, Wk, Wv, Wo, _trace=False):
    global LAST_RESULTS
    X = np.asarray(X, dtype=np.float32)
    S = X.shape[1]
    nc = _get_nc(S)
    in_maps = make_in_maps(X, np.asarray(Wq, np.float32), np.asarray(Wk, np.float32),
                           np.asarray(Wv, np.float32), np.asarray(Wo, np.float32))
    res = run_bass_kernel_spmd(nc, in_maps, list(range(N_CORES)), trace=_trace)
    LAST_RESULTS = res
    Y = np.zeros((B, S, D), dtype=np.float32)
    for i in range(N_CORES):
        Y[i // 4] += res.results[i]["yt"].T
    return Y


# revision 12
# speedup vs baseline: 1.2355x; 1.2355x over previous
"""Multi-head self-attention (B=2, S=4096, D=512, H=8, Dh=64) on 8 TRN2 cores.

Sharding: core i handles batch b = i//4 and head-pair hp = i%4 (heads 2*hp,
2*hp+1).  Each core computes Q/K/V projections for its two heads, flash-style
attention (no-max softmax; scores range is +-9 so exp is safe), and a partial
out-projection.  Host sums the 4 partial outputs per batch and transposes back.

v2 (bf16 pipeline): all matmul operands are bfloat16.  bf16 matmuls use
separate LDWEIGHTS instructions that the PE's 64-deep reorder window pulls
into the background weight buffer (plus FWL), so weight loads hide under the
streaming passes -- unlike fp32r matmuls, which self-load weights serially
(~107ns per matmul).  The kernel is restructured so the ACT engine (exp,
1 elem/cycle/lane @1.2GHz = the softmax roofline, ~290us for 33.5M exps)
never stalls:
  - scores PSUM tiles double-buffered, ctx PSUM double-buffered (normalize of
    block i runs under block i+1's compute; no PE idle -> no HAM re-throttle)
  - K projection accumulates chunk-by-chunk as the X DMA lands
  - out-projection of q-block i runs on the spare ctx-ring slot inside block
    i+1, copies on DVE (never ACT), output DMA overlapped
  - softmax normalize via reciprocal_approx_fast + DRAM-bounce partition
    broadcast, all off the critical path

Layouts (feature dim on partitions; every matmul contracts on partitions):
  xt  [512, S]  = X[b].T                       (bf16)
  wq/wk/wv [512, 128] = W[:, hp*128:(hp+1)*128] (bf16)
  wo  [128, 512] = Wo[hp*128:(hp+1)*128, :]     (bf16)
  yt  [512, S]  = partial (Y[b]).T              (fp32)

TRN2 quirk: walrus legalizes only ONE sync wait on TPB compute instructions.
`_legalize_matmul_waits` moves extra waits onto injected single-wait no-ops.
"""

import sys
from contextlib import ExitStack

for _p in ("/opt/trn_rl_repo",):
    if _p not in sys.path:
        sys.path.insert(0, _p)

import numpy as np

import concourse.bass as bass
import concourse.tile as tile
from concourse import mybir
from concourse.bass_utils import run_bass_kernel_spmd

F32 = mybir.dt.float32
BF16 = mybir.dt.bfloat16
MM_DT = BF16
D = 512          # model dim
DH = 64          # head dim
P = 128          # partitions
B = 2
H = 8
S_FULL = 4096
N_CORES = 8
NC_T = D // P    # 4 contraction tiles over model dim

LAST_RESULTS = None  # test harness reads exec_time_ns from here


def _emit(nc: bass.Bass, tc: "tile.TileContext", ctx: ExitStack, S: int):
    """Emit the per-core program. Parameterized by S for small-sim testing."""
    NK = S // P              # 128-row key tiles
    QB = 512                 # q-block (both heads processed per block)
    NQB = S // QB            # attention q-blocks
    inv_scale = 1.0 / np.sqrt(DH)

    def mm(out, lhsT, rhs, start=True, stop=True):
        return nc.tensor.matmul(out, lhsT, rhs, start=start, stop=stop)

    xt = nc.declare_dram_parameter("xt", [D, S], MM_DT, isOutput=False)
    wq = nc.declare_dram_parameter("wq", [D, P], MM_DT, isOutput=False)
    wk = nc.declare_dram_parameter("wk", [D, P], MM_DT, isOutput=False)
    wv = nc.declare_dram_parameter("wv", [D, P], MM_DT, isOutput=False)
    wo = nc.declare_dram_parameter("wo", [P, D], MM_DT, isOutput=False)
    yt = nc.declare_dram_parameter("yt", [D, S], F32, isOutput=True)

    const = ctx.enter_context(tc.tile_pool(name="const", bufs=1))

    # ---- weight DMA first (small), then X in [128,1024] pieces so the
    # projections that need only the first column-block can start early ----
    w_sb = {}
    for name, ap in (("wq", wq), ("wk", wk), ("wv", wv)):
        tiles = []
        for c in range(NC_T):
            t = const.tile([P, P], MM_DT, tag=f"{name}{c}", name=f"{name}{c}")
            nc.sync.dma_start(out=t[:], in_=ap[c * P:(c + 1) * P, :])
            tiles.append(t)
        w_sb[name] = tiles
    wo_sb = const.tile([P, D], MM_DT, tag="wo")
    nc.sync.dma_start(out=wo_sb[:], in_=wo[:, :])
    xt_sb = [const.tile([P, S], MM_DT, tag=f"xt{c}", name=f"xt{c}")
             for c in range(NC_T)]
    XPIECE = min(1024, S)
    for b in range(S // XPIECE):
        for c in range(NC_T):
            sl = slice(b * XPIECE, (b + 1) * XPIECE)
            nc.sync.dma_start(out=xt_sb[c][:, sl], in_=xt[c * P:(c + 1) * P, sl])

    # persistent intermediates
    qt_sb = const.tile([P, S], MM_DT, tag="qt")      # [2*64 d, S] stacked heads
    kt_sb = const.tile([P, S], MM_DT, tag="kt")
    # V with a ones column appended per k-tile: [128 k, NK*65]; col 64 == 1.0
    vones = [const.tile([P, NK * (DH + 1)], MM_DT, tag=f"vones{h}", name=f"vones{h}")
             for h in range(2)]
    konst = const.tile([P, NK, 1], F32, tag="konst")
    nc.vector.memset(konst[:], 1.0)
    for h in range(2):
        vv = vones[h].rearrange("p (k c) -> p k c", c=DH + 1)
        nc.vector.tensor_copy(vv[:, :, DH:DH + 1], konst[:])
    ctx_sb = const.tile([P, S], MM_DT, tag="ctx")    # context^T, stacked heads

    # PSUM banks: "s" 2 x [128,1024] (4) + "ctx" 3 x [65,512] (3) + "pp" 1 x
    # [128,512] (1) = 8.  Buffer *addresses* are assigned by autobufs
    # (interval packing); bufs= here are the per-tag caps.
    ps = ctx.enter_context(tc.tile_pool(name="ps", bufs=2, space="PSUM"))
    es = ctx.enter_context(tc.tile_pool(name="es", bufs=8))
    bcp = ctx.enter_context(tc.tile_pool(name="bcp", bufs=2))
    rtp = ctx.enter_context(tc.tile_pool(name="rtp", bufs=2))
    rdp = ctx.enter_context(tc.tile_pool(name="rdp", bufs=2, space="DRAM"))
    osb = ctx.enter_context(tc.tile_pool(name="osb", bufs=4))

    # ---- projections (tag "pp", 1 bank; they fill PE slack under the
    # ACT-bound attention loop, racing ahead of their consumers) ----
    def proj_block(dst, wname, lo):
        """dst[:, lo:lo+512] = (W^T x)[:, lo:lo+512] over the 4 chunks."""
        pq = ps.tile([P, 512], F32, tag="pp", bufs=1, name=f"p{wname}")
        for c in range(NC_T):
            mm(pq[:], w_sb[wname][c][:], xt_sb[c][:, lo:lo + 512],
               start=(c == 0), stop=(c == NC_T - 1))
        nc.vector.tensor_copy(dst[:, lo:lo + 512], pq[:])

    def proj_v(k):
        ksl = slice(k * P, (k + 1) * P)
        pv = ps.tile([P, P], F32, tag="pp", bufs=1, name="pv")
        for c in range(NC_T):
            mm(pv[:], xt_sb[c][:, ksl], w_sb["wv"][c][:],
               start=(c == 0), stop=(c == NC_T - 1))
        for h in range(2):
            nc.vector.tensor_copy(
                vones[h][:, k * (DH + 1):k * (DH + 1) + DH],
                pv[:, h * DH:(h + 1) * DH])

    # minimal upfront set: K/Q for the first q-block, first two V k-tiles
    NSB = S // 512
    proj_block(kt_sb, "wk", 0)
    if NSB > 1:
        proj_block(kt_sb, "wk", 512)
    proj_v(0)
    proj_v(1)
    proj_block(qt_sb, "wq", 0)
    # the rest, in consumption order (K block bp feeds score k-tiles 8bp+)
    for k in range(2, min(8, NK)):
        proj_v(k)
    for bp in range(2, NSB):
        proj_block(kt_sb, "wk", bp * 512)
        for k in range(4 * bp, 4 * bp + 4):
            proj_v(k)
    for bp in range(1, NSB):
        proj_block(qt_sb, "wq", bp * 512)

    # ---- phase B + C interleaved ----
    # Per (qb, k): the two heads' score matmuls are row-packed -- h0 uses PE
    # rows 0-63 (base_partition 0), h1 rows 64-127 (base_partition 64) -- and
    # run CONCURRENTLY in disjoint row-groups, writing the two 512-col halves
    # (= two different banks) of one [128,1024] PSUM tile.  A single N=1024
    # exp then covers both heads, keeping the ACT cadence at (1024+352)/1.2
    # ~= 1147ns per k-tile while PE streaming is only ~650ns (fits under the
    # ACT cadence even when the power manager halves the PE clock).
    def normalize(h, qb, ctx_ps):
        """Rowsum reciprocal + partition-broadcast for one head-block.
        The [1,QB] rowsum row is reshaped to [64, QB//64] via a DRAM bounce
        so the DVE reciprocal costs ~QB//64 columns, not QB.  Returns the
        [128, QB] broadcast tile of 1/rowsum (consumed ONLY by the DVE
        combine of the out-projection -- never gates the PE)."""
        NW = QB // DH        # columns per partition in the [64, NW] reshape
        rt = rtp.tile([1, QB], F32, tag="rt", name="rt")
        nc.vector.tensor_copy(rt[0:1, :], ctx_ps[DH:DH + 1, :])
        rd1 = rdp.tile([1, QB], F32, tag="rd1", name="rd1")
        nc.sync.dma_start(out=rd1[:], in_=rt[0:1, :])
        el = list(rd1[0:1, :].ap)[1]           # [elem_stride, QB]
        rd1_64 = bass.AP(tensor=rd1.tensor, offset=rd1.offset,
                         ap=[[el[0] * NW, DH], [el[0], NW]])
        r64 = rtp.tile([DH, NW], F32, tag="r64", name="r64")
        nc.sync.dma_start(out=r64[:], in_=rd1_64)
        rinv64 = rtp.tile([DH, NW], F32, tag="rinv64", name="rinv64")
        nc.vector.reciprocal(rinv64[:], r64[:])
        rd2 = rdp.tile([1, QB], F32, tag="rd2", name="rd2")
        el2 = list(rd2[0:1, :].ap)[1]
        rd2_64 = bass.AP(tensor=rd2.tensor, offset=rd2.offset,
                         ap=[[el2[0] * NW, DH], [el2[0], NW]])
        nc.sync.dma_start(out=rd2_64, in_=rinv64[:])
        rd2_bcast = bass.AP(tensor=rd2.tensor, offset=rd2.offset,
                            ap=[[0, P], el2])
        bc = bcp.tile([P, QB], F32, tag="bc", name="bc")
        nc.sync.dma_start(out=bc[:], in_=rd2_bcast)
        return bc

    def out_mm(prev, idx):
        """Out-projection tile idx for the PREVIOUS q-block: two K=64
        row-packed matmuls against the UNNORMALIZED context (ready right at
        the block boundary), staged out of PSUM immediately.  The 1/rowsum
        scales are applied later by out_combine on the DVE."""
        qsl, raws = prev["qsl"], []
        for h in range(2):
            hsl = slice(h * DH, (h + 1) * DH)
            o_ps = ps.tile([P, QB], F32, tag="pp", bufs=1, name="o_ps")
            mm(o_ps[:], wo_sb[hsl, idx * P:(idx + 1) * P], ctx_sb[hsl, qsl])
            o_raw = osb.tile([P, QB], F32, tag="oraw", bufs=4, name="o_raw")
            nc.vector.tensor_copy(o_raw[:], o_ps[:])
            raws.append(o_raw)
        prev["raw"][idx] = raws

    def out_combine(prev, idx):
        """o = o_h0 * bc_h0 + o_h1 * bc_h1, then DMA out."""
        r0, r1 = prev["raw"][idx]
        m0 = osb.tile([P, QB], F32, tag="m0", bufs=2, name="m0")
        nc.vector.tensor_mul(m0[:], r0[:], prev["bc"][0][:])
        m1 = osb.tile([P, QB], F32, tag="m1", bufs=2, name="m1")
        nc.vector.tensor_mul(m1[:], r1[:], prev["bc"][1][:])
        o_sb = osb.tile([P, QB], F32, tag="osb", bufs=2, name="o_sb")
        nc.vector.tensor_add(o_sb[:], m0[:], m1[:])
        nc.sync.dma_start(out=yt[idx * P:(idx + 1) * P, prev["qsl"]],
                          in_=o_sb[:])

    prev = None
    for qb in range(NQB):
        qsl = slice(qb * QB, (qb + 1) * QB)
        ctx_h = [ps.tile([DH + 1, QB], F32, tag="ctx", bufs=3,
                         name=f"ctx_ps{h}") for h in range(2)]
        for k in range(NK):
            s_pair = ps.tile([P, 2 * QB], F32, tag="s", name="s_pair")
            for h in range(2):
                hsl = slice(h * DH, (h + 1) * DH)
                mm(s_pair[:, h * QB:(h + 1) * QB],
                   kt_sb[hsl, k * P:(k + 1) * P], qt_sb[hsl, qsl])
            e_pair = es.tile([P, 2 * QB], MM_DT, tag="e", name="e_pair")
            nc.scalar.activation(e_pair[:], s_pair[:],
                                 mybir.ActivationFunctionType.Exp,
                                 scale=inv_scale)
            for h in range(2):
                vo = vones[h][:, k * (DH + 1):(k + 1) * (DH + 1)]
                mm(ctx_h[h][:], vo, e_pair[:, h * QB:(h + 1) * QB],
                   start=(k == 0), stop=(k == NK - 1))
            # previous q-block's out-projection, spread over this block's
            # PE/DVE slack: matmuls (vs unnormalized ctx -- no wait on the
            # normalize chain) at k=0..3, scale-combines at k=4..7
            if prev is not None:
                if k < NC_T:
                    out_mm(prev, k)
                elif k < 2 * NC_T:
                    out_combine(prev, k - NC_T)
        # stage the unnormalized context out of PSUM (bf16), then the
        # rowsum-reciprocal broadcast chains
        for h in range(2):
            nc.vector.tensor_copy(ctx_sb[h * DH:(h + 1) * DH, qsl],
                                  ctx_h[h][:DH, :])
        prev = {"qsl": qsl, "raw": [None] * NC_T,
                "bc": [normalize(h, qb, ctx_h[h]) for h in range(2)]}
    for idx in range(NC_T):
        out_mm(prev, idx)
    for idx in range(NC_T):
        out_combine(prev, idx)


_TPB_ENGINES = {mybir.EngineType.PE, mybir.EngineType.Activation,
                mybir.EngineType.DVE, mybir.EngineType.Pool}


def _legalize_matmul_waits(nc: bass.Bass) -> int:
    """Walrus encodes only ONE sync wait on TPB compute instructions (seen on
    Matmult and TensorCopy).  Move extra waits onto injected same-engine
    no-ops (one wait each) placed immediately before the instruction in its
    block: same semantics, legal encoding."""
    n_fixed = 0
    for f in nc.m.functions:
        for bb in f.blocks:
            out = []
            changed = False
            for ins in bb.instructions:
                si = ins.sync_info
                if (getattr(ins, "engine", None) is not None
                        and si is not None and len(si.on_wait) > 1):
                    for idx, w in enumerate(si.on_wait[:-1]):
                        nop = mybir.InstNoOp(name=f"{ins.name}-lgw{idx}",
                                             ins=[], outs=[])
                        nop.engine = ins.engine
                        nop.sync_info = mybir.SyncInfo(on_wait=[w], on_update=[])
                        out.append(nop)
                    ins.sync_info = mybir.SyncInfo(on_wait=[si.on_wait[-1]],
                                                   on_update=si.on_update)
                    n_fixed += 1
                    changed = True
                out.append(ins)
            if changed:
                bb.instructions = out
    return n_fixed


def build(S: int = S_FULL, legalize: bool = False) -> bass.Bass:
    nc = bass.Bass()
    with ExitStack() as ctx:
        ctx.enter_context(nc.allow_low_precision(
            reason="bf16 matmul operands / intermediates"))
        tc = ctx.enter_context(tile.TileContext(nc))
        _emit(nc, tc, ctx, S)
    if legalize:
        # only for the walrus/hardware path; CoreSim wants updates on every
        # instruction and doesn't enforce the 1-wait Matmult limit
        _legalize_matmul_waits(nc)
    return nc


_NC_CACHE = {}


def _get_nc(S: int) -> bass.Bass:
    if S not in _NC_CACHE:
        _NC_CACHE[S] = build(S, legalize=True)
    return _NC_CACHE[S]


def _bf16(a):
    import ml_dtypes
    return np.ascontiguousarray(np.asarray(a, dtype=np.float32)).astype(
        ml_dtypes.bfloat16)


def make_in_maps(X, Wq, Wk, Wv, Wo):
    X = np.asarray(X, dtype=np.float32)
    xts = [_bf16(X[b].T) for b in range(B)]
    in_maps = []
    for i in range(N_CORES):
        b, hp = divmod(i, 4)  # 4 head-pairs per batch
        csl = slice(hp * P, (hp + 1) * P)
        in_maps.append({
            "xt": xts[b],
            "wq": _bf16(Wq[:, csl]),
            "wk": _bf16(Wk[:, csl]),
            "wv": _bf16(Wv[:, csl]),
            "wo": _bf16(Wo[csl, :]),
        })
    return in_maps


def kernel(X, Wq, Wk, Wv, Wo, _trace=False):
    global LAST_RESULTS
    X = np.asarray(X, dtype=np.float32)
    S = X.shape[1]
    nc = _get_nc(S)
    in_maps = make_in_maps(X, np.asarray(Wq, np.float32), np.asarray(Wk, np.float32),
                           np.asarray(Wv, np.float32), np.asarray(Wo, np.float32))
    res = run_bass_kernel_spmd(nc, in_maps, list(range(N_CORES)), trace=_trace)
    LAST_RESULTS = res
    Y = np.zeros((B, S, D), dtype=np.float32)
    for i in range(N_CORES):
        Y[i // 4] += res.results[i]["yt"].T
    return Y


# revision 15
# speedup vs baseline: 1.2966x; 1.0495x over previous
"""Multi-head self-attention (B=2, S=4096, D=512, H=8, Dh=64) on 8 TRN2 cores.

Sharding: core i handles batch b = i//4 and head-pair hp = i%4 (heads 2*hp,
2*hp+1).  Each core computes Q/K/V projections for its two heads, flash-style
attention (no-max softmax; scores range is +-9 so exp is safe), and a partial
out-projection.  Host sums the 4 partial outputs per batch and transposes back.

v2 (bf16 pipeline): all matmul operands are bfloat16.  bf16 matmuls use
separate LDWEIGHTS instructions that the PE's 64-deep reorder window pulls
into the background weight buffer (plus FWL), so weight loads hide under the
streaming passes -- unlike fp32r matmuls, which self-load weights serially
(~107ns per matmul).  The kernel is restructured so the ACT engine (exp,
1 elem/cycle/lane @1.2GHz = the softmax roofline, ~290us for 33.5M exps)
never stalls:
  - scores PSUM tiles double-buffered, ctx PSUM double-buffered (normalize of
    block i runs under block i+1's compute; no PE idle -> no HAM re-throttle)
  - K projection accumulates chunk-by-chunk as the X DMA lands
  - out-projection of q-block i runs on the spare ctx-ring slot inside block
    i+1, copies on DVE (never ACT), output DMA overlapped
  - softmax normalize via reciprocal_approx_fast + DRAM-bounce partition
    broadcast, all off the critical path

Layouts (feature dim on partitions; every matmul contracts on partitions):
  xt  [512, S]  = X[b].T                       (bf16)
  wq/wk/wv [512, 128] = W[:, hp*128:(hp+1)*128] (bf16)
  wo  [128, 512] = Wo[hp*128:(hp+1)*128, :]     (bf16)
  yt  [512, S]  = partial (Y[b]).T              (fp32)

TRN2 quirk: walrus legalizes only ONE sync wait on TPB compute instructions.
`_legalize_matmul_waits` moves extra waits onto injected single-wait no-ops.
"""

import sys
from contextlib import ExitStack

for _p in ("/opt/trn_rl_repo",):
    if _p not in sys.path:
        sys.path.insert(0, _p)

import numpy as np

import concourse.bass as bass
import concourse.tile as tile
from concourse import mybir
from concourse.bass_utils import run_bass_kernel_spmd

F32 = mybir.dt.float32
BF16 = mybir.dt.bfloat16
MM_DT = BF16
D = 512          # model dim
DH = 64          # head dim
P = 128          # partitions
B = 2
H = 8
S_FULL = 4096
N_CORES = 8
NC_T = D // P    # 4 contraction tiles over model dim

LAST_RESULTS = None  # test harness reads exec_time_ns from here


def _emit(nc: bass.Bass, tc: "tile.TileContext", ctx: ExitStack, S: int):
    """Emit the per-core program. Parameterized by S for small-sim testing."""
    NK = S // P              # 128-row key tiles
    QB = 512                 # q-block (both heads processed per block)
    NQB = S // QB            # attention q-blocks
    inv_scale = 1.0 / np.sqrt(DH)

    def mm(out, lhsT, rhs, start=True, stop=True):
        return nc.tensor.matmul(out, lhsT, rhs, start=start, stop=stop)

    xt = nc.declare_dram_parameter("xt", [D, S], MM_DT, isOutput=False)
    wq = nc.declare_dram_parameter("wq", [D, P], MM_DT, isOutput=False)
    wk = nc.declare_dram_parameter("wk", [D, P], MM_DT, isOutput=False)
    wv = nc.declare_dram_parameter("wv", [D, P], MM_DT, isOutput=False)
    wo = nc.declare_dram_parameter("wo", [P, D], MM_DT, isOutput=False)
    yt = nc.declare_dram_parameter("yt", [D, S], F32, isOutput=True)

    const = ctx.enter_context(tc.tile_pool(name="const", bufs=1))

    # ---- weight DMA first (ONE dma each: a dma_start costs ~650ns of Sync
    # issue time regardless of size), then X in 8 pieces, first half first,
    # so the K projection for q-block 0 can start early ----
    w_sb = {}
    for name, ap in (("wk", wk), ("wq", wq), ("wv", wv)):
        w4 = const.tile([P, NC_T * P], MM_DT, tag=f"{name}4", name=f"{name}4")
        wap = ap[:, :]
        base = wap.ap          # [[row_stride, 512], [col_stride, 128]]
        src = bass.AP(tensor=wap.tensor, offset=wap.offset,
                      ap=[[base[0][0], P], [base[0][0] * P, NC_T],
                          [base[1][0], P]])
        nc.sync.dma_start(out=w4.rearrange("p (c j) -> p c j", c=NC_T), in_=src)
        w_sb[name] = [w4[:, c * P:(c + 1) * P] for c in range(NC_T)]
    wo_sb = const.tile([P, D], MM_DT, tag="wo")
    nc.sync.dma_start(out=wo_sb[:], in_=wo[:, :])
    xt_sb = [const.tile([P, S], MM_DT, tag=f"xt{c}", name=f"xt{c}")
             for c in range(NC_T)]
    XPIECE = min(2048, S)
    for b in range(S // XPIECE):
        for c in range(NC_T):
            sl = slice(b * XPIECE, (b + 1) * XPIECE)
            nc.sync.dma_start(out=xt_sb[c][:, sl], in_=xt[c * P:(c + 1) * P, sl])

    # persistent intermediates
    qt_sb = const.tile([P, S], MM_DT, tag="qt")      # [2*64 d, S] stacked heads
    kt_sb = const.tile([P, S], MM_DT, tag="kt")
    # V with a ones column appended per k-tile: [128 k, NK*65]; col 64 == 1.0
    vones = [const.tile([P, NK * (DH + 1)], MM_DT, tag=f"vones{h}", name=f"vones{h}")
             for h in range(2)]
    konst = const.tile([P, NK, 1], F32, tag="konst")
    nc.vector.memset(konst[:], 1.0)
    for h in range(2):
        vv = vones[h].rearrange("p (k c) -> p k c", c=DH + 1)
        nc.vector.tensor_copy(vv[:, :, DH:DH + 1], konst[:])
    ctx_sb = const.tile([P, S], MM_DT, tag="ctx")    # context^T, stacked heads

    # PSUM banks: "s" 2 x [128,1024] (4) + "ctx" 2 x [65,512] (2) + "pp" 2 x
    # [128,512] (2) = 8.  Buffer *addresses* are assigned by autobufs
    # (interval packing); bufs= here are the per-tag caps.
    ps = ctx.enter_context(tc.tile_pool(name="ps", bufs=2, space="PSUM"))
    es = ctx.enter_context(tc.tile_pool(name="es", bufs=8))
    bcp = ctx.enter_context(tc.tile_pool(name="bcp", bufs=2))
    rtp = ctx.enter_context(tc.tile_pool(name="rtp", bufs=2))
    rdp = ctx.enter_context(tc.tile_pool(name="rdp", bufs=2, space="DRAM"))
    osb = ctx.enter_context(tc.tile_pool(name="osb", bufs=4))

    # ---- projections (tag "pp", 1 bank; they fill PE slack under the
    # ACT-bound attention loop, racing ahead of their consumers) ----
    def proj_block(dst, wname, lo):
        """dst[:, lo:lo+512] = (W^T x)[:, lo:lo+512] over the 4 chunks."""
        pq = ps.tile([P, 512], F32, tag="pp", bufs=2, name=f"p{wname}")
        for c in range(NC_T):
            mm(pq[:], w_sb[wname][c][:], xt_sb[c][:, lo:lo + 512],
               start=(c == 0), stop=(c == NC_T - 1))
        nc.vector.tensor_copy(dst[:, lo:lo + 512], pq[:])

    def proj_v(k):
        ksl = slice(k * P, (k + 1) * P)
        pv = ps.tile([P, P], F32, tag="pp", bufs=2, name="pv")
        for c in range(NC_T):
            mm(pv[:], xt_sb[c][:, ksl], w_sb["wv"][c][:],
               start=(c == 0), stop=(c == NC_T - 1))
        for h in range(2):
            nc.vector.tensor_copy(
                vones[h][:, k * (DH + 1):k * (DH + 1) + DH],
                pv[:, h * DH:(h + 1) * DH])

    # minimal upfront set: K/Q for the first q-block, first two V k-tiles
    NSB = S // 512
    proj_block(kt_sb, "wk", 0)
    if NSB > 1:
        proj_block(kt_sb, "wk", 512)
    proj_v(0)
    proj_v(1)
    proj_block(qt_sb, "wq", 0)
    # the rest, in consumption order (K block bp feeds score k-tiles 8bp+)
    for k in range(2, min(8, NK)):
        proj_v(k)
    for bp in range(2, NSB):
        proj_block(kt_sb, "wk", bp * 512)
        for k in range(4 * bp, 4 * bp + 4):
            proj_v(k)
    for bp in range(1, NSB):
        proj_block(qt_sb, "wq", bp * 512)

    # ---- phase B + C interleaved ----
    # Per (qb, k): the two heads' score matmuls are row-packed -- h0 uses PE
    # rows 0-63 (base_partition 0), h1 rows 64-127 (base_partition 64) -- and
    # run CONCURRENTLY in disjoint row-groups, writing the two 512-col halves
    # (= two different banks) of one [128,1024] PSUM tile.  A single N=1024
    # exp then covers both heads, keeping the ACT cadence at (1024+352)/1.2
    # ~= 1147ns per k-tile while PE streaming is only ~650ns (fits under the
    # ACT cadence even when the power manager halves the PE clock).
    def normalize(h, qb, ctx_ps):
        """Rowsum reciprocal + partition-broadcast for one head-block.
        The [1,QB] rowsum row is reshaped to [64, QB//64] via a DRAM bounce
        so the DVE reciprocal costs ~QB//64 columns, not QB.  Returns the
        [128, QB] broadcast tile of 1/rowsum (consumed ONLY by the DVE
        combine of the out-projection -- never gates the PE)."""
        NW = QB // DH        # columns per partition in the [64, NW] reshape
        rt = rtp.tile([1, QB], F32, tag="rt", name="rt")
        nc.vector.tensor_copy(rt[0:1, :], ctx_ps[DH:DH + 1, :])
        rd1 = rdp.tile([1, QB], F32, tag="rd1", name="rd1")
        nc.sync.dma_start(out=rd1[:], in_=rt[0:1, :])
        el = list(rd1[0:1, :].ap)[1]           # [elem_stride, QB]
        rd1_64 = bass.AP(tensor=rd1.tensor, offset=rd1.offset,
                         ap=[[el[0] * NW, DH], [el[0], NW]])
        r64 = rtp.tile([DH, NW], F32, tag="r64", name="r64")
        nc.sync.dma_start(out=r64[:], in_=rd1_64)
        rinv64 = rtp.tile([DH, NW], F32, tag="rinv64", name="rinv64")
        nc.vector.reciprocal(rinv64[:], r64[:])
        rd2 = rdp.tile([1, QB], F32, tag="rd2", name="rd2")
        el2 = list(rd2[0:1, :].ap)[1]
        rd2_64 = bass.AP(tensor=rd2.tensor, offset=rd2.offset,
                         ap=[[el2[0] * NW, DH], [el2[0], NW]])
        nc.sync.dma_start(out=rd2_64, in_=rinv64[:])
        rd2_bcast = bass.AP(tensor=rd2.tensor, offset=rd2.offset,
                            ap=[[0, P], el2])
        bc = bcp.tile([P, QB], F32, tag="bc", name="bc")
        nc.sync.dma_start(out=bc[:], in_=rd2_bcast)
        return bc

    def out_mm(prev, idx):
        """Out-projection tile idx for the PREVIOUS q-block: two K=64
        row-packed matmuls against the UNNORMALIZED context (ready right at
        the block boundary), staged out of PSUM immediately.  The 1/rowsum
        scales are applied later by out_combine on the DVE."""
        qsl, raws = prev["qsl"], []
        for h in range(2):
            hsl = slice(h * DH, (h + 1) * DH)
            o_ps = ps.tile([P, QB], F32, tag="pp", bufs=2, name="o_ps")
            mm(o_ps[:], wo_sb[hsl, idx * P:(idx + 1) * P], ctx_sb[hsl, qsl])
            o_raw = osb.tile([P, QB], F32, tag="oraw", bufs=4, name="o_raw")
            nc.vector.tensor_copy(o_raw[:], o_ps[:])
            raws.append(o_raw)
        prev["raw"][idx] = raws

    def out_combine(prev, idx):
        """o = o_h0 * bc_h0 + o_h1 * bc_h1, then DMA out."""
        r0, r1 = prev["raw"][idx]
        m0 = osb.tile([P, QB], F32, tag="m0", bufs=2, name="m0")
        nc.vector.tensor_mul(m0[:], r0[:], prev["bc"][0][:])
        m1 = osb.tile([P, QB], F32, tag="m1", bufs=2, name="m1")
        nc.vector.tensor_mul(m1[:], r1[:], prev["bc"][1][:])
        o_sb = osb.tile([P, QB], F32, tag="osb", bufs=2, name="o_sb")
        nc.vector.tensor_add(o_sb[:], m0[:], m1[:])
        nc.sync.dma_start(out=yt[idx * P:(idx + 1) * P, prev["qsl"]],
                          in_=o_sb[:])

    prev = None
    for qb in range(NQB):
        qsl = slice(qb * QB, (qb + 1) * QB)
        ctx_h = [ps.tile([DH + 1, QB], F32, tag="ctx", bufs=2,
                         name=f"ctx_ps{h}") for h in range(2)]
        for k in range(NK):
            s_pair = ps.tile([P, 2 * QB], F32, tag="s", name="s_pair")
            for h in range(2):
                hsl = slice(h * DH, (h + 1) * DH)
                mm(s_pair[:, h * QB:(h + 1) * QB],
                   kt_sb[hsl, k * P:(k + 1) * P], qt_sb[hsl, qsl])
            e_pair = es.tile([P, 2 * QB], MM_DT, tag="e", name="e_pair")
            nc.scalar.activation(e_pair[:], s_pair[:],
                                 mybir.ActivationFunctionType.Exp,
                                 scale=inv_scale)
            for h in range(2):
                vo = vones[h][:, k * (DH + 1):(k + 1) * (DH + 1)]
                mm(ctx_h[h][:], vo, e_pair[:, h * QB:(h + 1) * QB],
                   start=(k == 0), stop=(k == NK - 1))
            # previous q-block's out-projection, spread over this block's
            # PE/DVE slack: matmuls (vs unnormalized ctx -- no wait on the
            # normalize chain) at k=0..3, scale-combines at k=4..7
            if prev is not None:
                if k < NC_T:
                    out_mm(prev, k)
                elif k < 2 * NC_T:
                    out_combine(prev, k - NC_T)
        # stage the unnormalized context out of PSUM (bf16), then the
        # rowsum-reciprocal broadcast chains
        for h in range(2):
            nc.vector.tensor_copy(ctx_sb[h * DH:(h + 1) * DH, qsl],
                                  ctx_h[h][:DH, :])
        prev = {"qsl": qsl, "raw": [None] * NC_T,
                "bc": [normalize(h, qb, ctx_h[h]) for h in range(2)]}
    for idx in range(NC_T):
        out_mm(prev, idx)
    for idx in range(NC_T):
        out_combine(prev, idx)


_TPB_ENGINES = {mybir.EngineType.PE, mybir.EngineType.Activation,
                mybir.EngineType.DVE, mybir.EngineType.Pool}


def _legalize_matmul_waits(nc: bass.Bass) -> int:
    """Walrus encodes only ONE sync wait on TPB compute instructions (seen on
    Matmult and TensorCopy).  Move extra waits onto injected same-engine
    no-ops (one wait each) placed immediately before the instruction in its
    block: same semantics, legal encoding."""
    n_fixed = 0
    for f in nc.m.functions:
        for bb in f.blocks:
            out = []
            changed = False
            for ins in bb.instructions:
                si = ins.sync_info
                if (getattr(ins, "engine", None) is not None
                        and si is not None and len(si.on_wait) > 1):
                    for idx, w in enumerate(si.on_wait[:-1]):
                        nop = mybir.InstNoOp(name=f"{ins.name}-lgw{idx}",
                                             ins=[], outs=[])
                        nop.engine = ins.engine
                        nop.sync_info = mybir.SyncInfo(on_wait=[w], on_update=[])
                        out.append(nop)
                    ins.sync_info = mybir.SyncInfo(on_wait=[si.on_wait[-1]],
                                                   on_update=si.on_update)
                    n_fixed += 1
                    changed = True
                out.append(ins)
            if changed:
                bb.instructions = out
    return n_fixed


def build(S: int = S_FULL, legalize: bool = False) -> bass.Bass:
    nc = bass.Bass()
    with ExitStack() as ctx:
        ctx.enter_context(nc.allow_low_precision(
            reason="bf16 matmul operands / intermediates"))
        tc = ctx.enter_context(tile.TileContext(nc))
        _emit(nc, tc, ctx, S)
    if legalize:
        # only for the walrus/hardware path; CoreSim wants updates on every
        # instruction and doesn't enforce the 1-wait Matmult limit
        _legalize_matmul_waits(nc)
    return nc


_NC_CACHE = {}


def _get_nc(S: int) -> bass.Bass:
    if S not in _NC_CACHE:
        _NC_CACHE[S] = build(S, legalize=True)
    return _NC_CACHE[S]


def _bf16(a):
    import ml_dtypes
    return np.ascontiguousarray(np.asarray(a, dtype=np.float32)).astype(
        ml_dtypes.bfloat16)


def make_in_maps(X, Wq, Wk, Wv, Wo):
    X = np.asarray(X, dtype=np.float32)
    xts = [_bf16(X[b].T) for b in range(B)]
    in_maps = []
    for i in range(N_CORES):
        b, hp = divmod(i, 4)  # 4 head-pairs per batch
        csl = slice(hp * P, (hp + 1) * P)
        in_maps.append({
            "xt": xts[b],
            "wq": _bf16(Wq[:, csl]),
            "wk": _bf16(Wk[:, csl]),
            "wv": _bf16(Wv[:, csl]),
            "wo": _bf16(Wo[csl, :]),
        })
    return in_maps


def kernel(X, Wq, Wk, Wv, Wo, _trace=False):
    global LAST_RESULTS
    X = np.asarray(X, dtype=np.float32)
    S = X.shape[1]
    nc = _get_nc(S)
    in_maps = make_in_maps(X, np.asarray(Wq, np.float32), np.asarray(Wk, np.float32),
                           np.asarray(Wv, np.float32), np.asarray(Wo, np.float32))
    res = run_bass_kernel_spmd(nc, in_maps, list(range(N_CORES)), trace=_trace)
    LAST_RESULTS = res
    Y = np.zeros((B, S, D), dtype=np.float32)
    for i in range(N_CORES):
        Y[i // 4] += res.results[i]["yt"].T
    return Y


# revision 19
# speedup vs baseline: 1.3218x; 1.0194x over previous
"""Multi-head self-attention (B=2, S=4096, D=512, H=8, Dh=64) on 8 TRN2 cores.

Sharding: core i handles batch b = i//4 and head-pair hp = i%4 (heads 2*hp,
2*hp+1).  Each core computes Q/K/V projections for its two heads, flash-style
attention (no-max softmax; scores range is +-9 so exp is safe), and a partial
out-projection.  Host sums the 4 partial outputs per batch and transposes back.

v2 (bf16 pipeline): all matmul operands are bfloat16.  bf16 matmuls use
separate LDWEIGHTS instructions that the PE's 64-deep reorder window pulls
into the background weight buffer (plus FWL), so weight loads hide under the
streaming passes -- unlike fp32r matmuls, which self-load weights serially
(~107ns per matmul).  The kernel is restructured so the ACT engine (exp,
1 elem/cycle/lane @1.2GHz = the softmax roofline, ~290us for 33.5M exps)
never stalls:
  - scores PSUM tiles double-buffered, ctx PSUM double-buffered (normalize of
    block i runs under block i+1's compute; no PE idle -> no HAM re-throttle)
  - K projection accumulates chunk-by-chunk as the X DMA lands
  - out-projection of q-block i runs on the spare ctx-ring slot inside block
    i+1, copies on DVE (never ACT), output DMA overlapped
  - softmax normalize via reciprocal_approx_fast + DRAM-bounce partition
    broadcast, all off the critical path

Layouts (feature dim on partitions; every matmul contracts on partitions):
  xt  [512, S]  = X[b].T                       (bf16)
  wq/wk/wv [512, 128] = W[:, hp*128:(hp+1)*128] (bf16)
  wo  [128, 512] = Wo[hp*128:(hp+1)*128, :]     (bf16)
  yt  [512, S]  = partial (Y[b]).T              (fp32)

TRN2 quirk: walrus legalizes only ONE sync wait on TPB compute instructions.
`_legalize_matmul_waits` moves extra waits onto injected single-wait no-ops.
"""

import sys
from contextlib import ExitStack

for _p in ("/opt/trn_rl_repo",):
    if _p not in sys.path:
        sys.path.insert(0, _p)

import numpy as np

import concourse.bass as bass
import concourse.tile as tile
from concourse import mybir
from concourse.bass_utils import run_bass_kernel_spmd
from concourse.masks import make_identity

F32 = mybir.dt.float32
BF16 = mybir.dt.bfloat16
MM_DT = BF16
D = 512          # model dim
DH = 64          # head dim
P = 128          # partitions
B = 2
H = 8
S_FULL = 4096
N_CORES = 8
NC_T = D // P    # 4 contraction tiles over model dim

LAST_RESULTS = None  # test harness reads exec_time_ns from here


def _emit(nc: bass.Bass, tc: "tile.TileContext", ctx: ExitStack, S: int):
    """Emit the per-core program. Parameterized by S for small-sim testing."""
    NK = S // P              # 128-row key tiles
    QB = 512                 # q-block (both heads processed per block)
    NQB = S // QB            # attention q-blocks
    inv_scale = 1.0 / np.sqrt(DH)

    def mm(out, lhsT, rhs, start=True, stop=True):
        return nc.tensor.matmul(out, lhsT, rhs, start=start, stop=stop)

    xt = nc.declare_dram_parameter("xt", [D, S], MM_DT, isOutput=False)
    wq = nc.declare_dram_parameter("wq", [D, P], MM_DT, isOutput=False)
    wk = nc.declare_dram_parameter("wk", [D, P], MM_DT, isOutput=False)
    wv = nc.declare_dram_parameter("wv", [D, P], MM_DT, isOutput=False)
    wo = nc.declare_dram_parameter("wo", [P, D], MM_DT, isOutput=False)
    yt = nc.declare_dram_parameter("yt", [D, S], F32, isOutput=True)

    const = ctx.enter_context(tc.tile_pool(name="const", bufs=1))

    # ---- weight DMA first (ONE dma each: a dma_start costs ~650ns of Sync
    # issue time regardless of size), then X in 8 pieces, first half first,
    # so the K projection for q-block 0 can start early ----
    w_sb = {}
    for name, ap in (("wk", wk), ("wq", wq), ("wv", wv)):
        w4 = const.tile([P, NC_T * P], MM_DT, tag=f"{name}4", name=f"{name}4")
        wap = ap[:, :]
        base = wap.ap          # [[row_stride, 512], [col_stride, 128]]
        src = bass.AP(tensor=wap.tensor, offset=wap.offset,
                      ap=[[base[0][0], P], [base[0][0] * P, NC_T],
                          [base[1][0], P]])
        nc.sync.dma_start(out=w4.rearrange("p (c j) -> p c j", c=NC_T), in_=src)
        w_sb[name] = [w4[:, c * P:(c + 1) * P] for c in range(NC_T)]
    wo_sb = const.tile([P, D], MM_DT, tag="wo")
    nc.sync.dma_start(out=wo_sb[:], in_=wo[:, :])
    xt_sb = [const.tile([P, S], MM_DT, tag=f"xt{c}", name=f"xt{c}")
             for c in range(NC_T)]
    XPIECE = min(2048, S)
    for b in range(S // XPIECE):
        for c in range(NC_T):
            sl = slice(b * XPIECE, (b + 1) * XPIECE)
            nc.sync.dma_start(out=xt_sb[c][:, sl], in_=xt[c * P:(c + 1) * P, sl])

    # persistent intermediates
    qt_sb = const.tile([P, S], MM_DT, tag="qt")      # [2*64 d, S] stacked heads
    kt_sb = const.tile([P, S], MM_DT, tag="kt")
    # V with a ones column appended per k-tile: [128 k, NK*65]; col 64 == 1.0
    vones = [const.tile([P, NK * (DH + 1)], MM_DT, tag=f"vones{h}", name=f"vones{h}")
             for h in range(2)]
    konst = const.tile([P, NK, 1], F32, tag="konst")
    nc.vector.memset(konst[:], 1.0)
    for h in range(2):
        vv = vones[h].rearrange("p (k c) -> p k c", c=DH + 1)
        nc.vector.tensor_copy(vv[:, :, DH:DH + 1], konst[:])
    ctx_sb = const.tile([P, S], MM_DT, tag="ctx")    # context^T, stacked heads

    # PSUM banks: "s" 2 x [128,1024] (4) + "ctx" 2 x [65,512] (2) + "pp" 2 x
    # [128,512] (2) = 8.  Buffer *addresses* are assigned by autobufs
    # (interval packing); bufs= here are the per-tag caps.
    ps = ctx.enter_context(tc.tile_pool(name="ps", bufs=2, space="PSUM"))
    es = ctx.enter_context(tc.tile_pool(name="es", bufs=8))
    bcp = ctx.enter_context(tc.tile_pool(name="bcp", bufs=2))
    rtp = ctx.enter_context(tc.tile_pool(name="rtp", bufs=2))
    rdp = ctx.enter_context(tc.tile_pool(name="rdp", bufs=2, space="DRAM"))
    osb = ctx.enter_context(tc.tile_pool(name="osb", bufs=4))
    vtp = ctx.enter_context(tc.tile_pool(name="vtp", bufs=2))

    ident = const.tile([P, P], MM_DT, tag="ident")
    make_identity(nc, ident[:])

    # ---- projections (tag "pp"; they fill PE slack under the ACT-bound
    # attention loop, racing ahead of their consumers) ----
    def proj_block(dst, wname, lo):
        """dst[:, lo:lo+512] = (W^T x)[:, lo:lo+512] over the 4 chunks."""
        pq = ps.tile([P, 512], F32, tag="pp", bufs=2, name=f"p{wname}")
        for c in range(NC_T):
            mm(pq[:], w_sb[wname][c][:], xt_sb[c][:, lo:lo + 512],
               start=(c == 0), stop=(c == NC_T - 1))
        nc.vector.tensor_copy(dst[:, lo:lo + 512], pq[:])

    def proj_v_group(g):
        """V for k-tiles 4g..4g+3, v-major N=512 matmuls (one stationary per
        chunk instead of per k-tile -- V at N=128 is LDWEIGHTS-bound), then
        PE-transposed back to k-major for the vones layout."""
        sl = slice(g * 512, (g + 1) * 512)
        pv = ps.tile([P, 512], F32, tag="pp", bufs=2, name="pv")
        for c in range(NC_T):
            mm(pv[:], w_sb["wv"][c][:], xt_sb[c][:, sl],
               start=(c == 0), stop=(c == NC_T - 1))
        vtmp = vtp.tile([P, 512], MM_DT, tag="vt", name="vtmp")
        nc.vector.tensor_copy(vtmp[:], pv[:])
        pt = ps.tile([P, 512], MM_DT, tag="pp", bufs=2, name="pt")
        for j in range(4):
            nc.tensor.transpose(pt[:, j * P:(j + 1) * P],
                                vtmp[:, j * P:(j + 1) * P], ident[:])
        for j in range(4):
            k = 4 * g + j
            for h in range(2):
                nc.vector.tensor_copy(
                    vones[h][:, k * (DH + 1):k * (DH + 1) + DH],
                    pt[:, j * P + h * DH:j * P + (h + 1) * DH])

    # upfront: K for q-block 0, first V group, Q for q-block 0; the rest
    # races the attention loop (Q blocks are emitted inside the loop, 4
    # k-iterations before their consumer)
    NSB = S // 512
    proj_block(kt_sb, "wk", 0)
    if NSB > 1:
        proj_block(kt_sb, "wk", 512)
    proj_v_group(0)
    proj_block(qt_sb, "wq", 0)
    for g in range(1, NSB):
        if g >= 2:
            proj_block(kt_sb, "wk", g * 512)
        proj_v_group(g)

    # ---- phase B + C interleaved ----
    # Per (qb, k): the two heads' score matmuls are row-packed -- h0 uses PE
    # rows 0-63 (base_partition 0), h1 rows 64-127 (base_partition 64) -- and
    # run CONCURRENTLY in disjoint row-groups, writing the two 512-col halves
    # (= two different banks) of one [128,1024] PSUM tile.  A single N=1024
    # exp then covers both heads, keeping the ACT cadence at (1024+352)/1.2
    # ~= 1147ns per k-tile while PE streaming is only ~650ns (fits under the
    # ACT cadence even when the power manager halves the PE clock).
    def normalize(h, qb, ctx_ps):
        """Rowsum reciprocal + partition-broadcast for one head-block.
        The [1,QB] rowsum row is reshaped to [64, QB//64] via a DRAM bounce
        so the DVE reciprocal costs ~QB//64 columns, not QB.  Returns the
        [128, QB] broadcast tile of 1/rowsum (consumed ONLY by the DVE
        combine of the out-projection -- never gates the PE)."""
        NW = QB // DH        # columns per partition in the [64, NW] reshape
        rt = rtp.tile([1, QB], F32, tag="rt", name="rt")
        nc.vector.tensor_copy(rt[0:1, :], ctx_ps[DH:DH + 1, :])
        rd1 = rdp.tile([1, QB], F32, tag="rd1", name="rd1")
        nc.sync.dma_start(out=rd1[:], in_=rt[0:1, :])
        el = list(rd1[0:1, :].ap)[1]           # [elem_stride, QB]
        rd1_64 = bass.AP(tensor=rd1.tensor, offset=rd1.offset,
                         ap=[[el[0] * NW, DH], [el[0], NW]])
        r64 = rtp.tile([DH, NW], F32, tag="r64", name="r64")
        nc.sync.dma_start(out=r64[:], in_=rd1_64)
        rinv64 = rtp.tile([DH, NW], F32, tag="rinv64", name="rinv64")
        nc.vector.reciprocal(rinv64[:], r64[:])
        rd2 = rdp.tile([1, QB], F32, tag="rd2", name="rd2")
        el2 = list(rd2[0:1, :].ap)[1]
        rd2_64 = bass.AP(tensor=rd2.tensor, offset=rd2.offset,
                         ap=[[el2[0] * NW, DH], [el2[0], NW]])
        nc.sync.dma_start(out=rd2_64, in_=rinv64[:])
        rd2_bcast = bass.AP(tensor=rd2.tensor, offset=rd2.offset,
                            ap=[[0, P], el2])
        bc = bcp.tile([P, QB], F32, tag="bc", name="bc")
        nc.sync.dma_start(out=bc[:], in_=rd2_bcast)
        return bc

    def out_mm(prev, idx, tag="pp"):
        """Out-projection tile idx for the PREVIOUS q-block: two K=64
        row-packed matmuls against the UNNORMALIZED context (ready right at
        the block boundary), staged out of PSUM immediately.  The 1/rowsum
        scales are applied later by out_combine on the DVE."""
        qsl, raws = prev["qsl"], []
        for h in range(2):
            hsl = slice(h * DH, (h + 1) * DH)
            o_ps = ps.tile([P, QB], F32, tag=tag, bufs=2, name="o_ps")
            mm(o_ps[:], wo_sb[hsl, idx * P:(idx + 1) * P], ctx_sb[hsl, qsl])
            o_raw = osb.tile([P, QB], F32, tag="oraw", bufs=4, name="o_raw")
            nc.vector.tensor_copy(o_raw[:], o_ps[:])
            raws.append(o_raw)
        prev["raw"][idx] = raws

    def out_combine(prev, idx):
        """o = o_h0 * bc_h0 + o_h1 * bc_h1, then DMA out.  One mul rides the
        (otherwise idle) GpSimd engine."""
        r0, r1 = prev["raw"][idx]
        m0 = osb.tile([P, QB], F32, tag="m0", bufs=2, name="m0")
        nc.vector.tensor_mul(m0[:], r0[:], prev["bc"][0][:])
        m1 = osb.tile([P, QB], F32, tag="m1", bufs=2, name="m1")
        nc.gpsimd.tensor_mul(m1[:], r1[:], prev["bc"][1][:])
        o_sb = osb.tile([P, QB], F32, tag="osb", bufs=2, name="o_sb")
        nc.vector.tensor_add(o_sb[:], m0[:], m1[:])
        nc.sync.dma_start(out=yt[idx * P:(idx + 1) * P, prev["qsl"]],
                          in_=o_sb[:])

    def emit_scores(qb, k):
        qsl = slice(qb * QB, (qb + 1) * QB)
        s_pair = ps.tile([P, 2 * QB], F32, tag="s", name="s_pair")
        for h in range(2):
            hsl = slice(h * DH, (h + 1) * DH)
            mm(s_pair[:, h * QB:(h + 1) * QB],
               kt_sb[hsl, k * P:(k + 1) * P], qt_sb[hsl, qsl])
        return s_pair

    # Software-pipelined main loop: iteration i+1's score matmuls are
    # emitted (= prioritized) ahead of iteration i's ctx matmuls, so at a
    # q-block boundary the next block's first scores run before the old
    # block's final ctx pair and the exp stream never waits.
    iters = [(qb, k) for qb in range(NQB) for k in range(NK)]
    ctx_blocks = {}
    prev = None
    spair_next = emit_scores(0, 0)
    for i, (qb, k) in enumerate(iters):
        qsl = slice(qb * QB, (qb + 1) * QB)
        if k == 0:
            ctx_blocks[qb] = [ps.tile([DH + 1, QB], F32, tag="ctx", bufs=2,
                                      name=f"ctx_ps{h}") for h in range(2)]
        s_pair, ctx_h = spair_next, ctx_blocks[qb]
        if i + 1 < len(iters):
            spair_next = emit_scores(*iters[i + 1])
        e_pair = es.tile([P, 2 * QB], MM_DT, tag="e", name="e_pair")
        nc.scalar.activation(e_pair[:], s_pair[:],
                             mybir.ActivationFunctionType.Exp,
                             scale=inv_scale)
        for h in range(2):
            vo = vones[h][:, k * (DH + 1):(k + 1) * (DH + 1)]
            mm(ctx_h[h][:], vo, e_pair[:, h * QB:(h + 1) * QB],
               start=(k == 0), stop=(k == NK - 1))
        # previous q-block's out-projection, spread over this block's
        # PE/DVE slack: matmuls (vs unnormalized ctx -- no wait on the
        # normalize chain) at k=0..3, scale-combines at k=4..7
        if prev is not None:
            if k < NC_T:
                out_mm(prev, k)
            elif k < 2 * NC_T:
                out_combine(prev, k - NC_T)
        # next q-block's Q projection, 4 iterations before its consumer
        # (the score-ahead emission at k == NK-1)
        if k == NK - 4 and qb + 1 < NQB:
            proj_block(qt_sb, "wq", (qb + 1) * 512)
        if k == NK - 1:
            # stage the unnormalized context out of PSUM (bf16), then the
            # rowsum-reciprocal broadcast chains
            for h in range(2):
                nc.vector.tensor_copy(ctx_sb[h * DH:(h + 1) * DH, qsl],
                                      ctx_h[h][:DH, :])
            prev = {"qsl": qsl, "raw": [None] * NC_T,
                    "bc": [normalize(h, qb, ctx_blocks[qb][h])
                           for h in range(2)]}
    for idx in range(NC_T):
        out_mm(prev, idx, tag="s")
    for idx in range(NC_T):
        out_combine(prev, idx)


_TPB_ENGINES = {mybir.EngineType.PE, mybir.EngineType.Activation,
                mybir.EngineType.DVE, mybir.EngineType.Pool}


def _legalize_matmul_waits(nc: bass.Bass) -> int:
    """Walrus encodes only ONE sync wait on TPB compute instructions (seen on
    Matmult and TensorCopy).  Move extra waits onto injected same-engine
    no-ops (one wait each) placed immediately before the instruction in its
    block: same semantics, legal encoding."""
    n_fixed = 0
    for f in nc.m.functions:
        for bb in f.blocks:
            out = []
            changed = False
            for ins in bb.instructions:
                si = ins.sync_info
                if (getattr(ins, "engine", None) is not None
                        and si is not None and len(si.on_wait) > 1):
                    for idx, w in enumerate(si.on_wait[:-1]):
                        nop = mybir.InstNoOp(name=f"{ins.name}-lgw{idx}",
                                             ins=[], outs=[])
                        nop.engine = ins.engine
                        nop.sync_info = mybir.SyncInfo(on_wait=[w], on_update=[])
                        out.append(nop)
                    ins.sync_info = mybir.SyncInfo(on_wait=[si.on_wait[-1]],
                                                   on_update=si.on_update)
                    n_fixed += 1
                    changed = True
                out.append(ins)
            if changed:
                bb.instructions = out
    return n_fixed


def build(S: int = S_FULL, legalize: bool = False) -> bass.Bass:
    nc = bass.Bass()
    with ExitStack() as ctx:
        ctx.enter_context(nc.allow_low_precision(
            reason="bf16 matmul operands / intermediates"))
        tc = ctx.enter_context(tile.TileContext(nc))
        _emit(nc, tc, ctx, S)
    if legalize:
        # only for the walrus/hardware path; CoreSim wants updates on every
        # instruction and doesn't enforce the 1-wait Matmult limit
        _legalize_matmul_waits(nc)
    return nc


_NC_CACHE = {}


def _get_nc(S: int) -> bass.Bass:
    if S not in _NC_CACHE:
        _NC_CACHE[S] = build(S, legalize=True)
    return _NC_CACHE[S]


def _bf16(a):
    import ml_dtypes
    return np.ascontiguousarray(np.asarray(a, dtype=np.float32)).astype(
        ml_dtypes.bfloat16)


def make_in_maps(X, Wq, Wk, Wv, Wo):
    X = np.asarray(X, dtype=np.float32)
    xts = [_bf16(X[b].T) for b in range(B)]
    in_maps = []
    for i in range(N_CORES):
        b, hp = divmod(i, 4)  # 4 head-pairs per batch
        csl = slice(hp * P, (hp + 1) * P)
        in_maps.append({
            "xt": xts[b],
            "wq": _bf16(Wq[:, csl]),
            "wk": _bf16(Wk[:, csl]),
            "wv": _bf16(Wv[:, csl]),
            "wo": _bf16(Wo[csl, :]),
        })
    return in_maps


def kernel(X, Wq, Wk, Wv, Wo, _trace=False):
    global LAST_RESULTS
    X = np.asarray(X, dtype=np.float32)
    S = X.shape[1]
    nc = _get_nc(S)
    in_maps = make_in_maps(X, np.asarray(Wq, np.float32), np.asarray(Wk, np.float32),
                           np.asarray(Wv, np.float32), np.asarray(Wo, np.float32))
    res = run_bass_kernel_spmd(nc, in_maps, list(range(N_CORES)), trace=_trace)
    LAST_RESULTS = res
    Y = np.zeros((B, S, D), dtype=np.float32)
    for i in range(N_CORES):
        Y[i // 4] += res.results[i]["yt"].T
    return Y


# revision 22
# speedup vs baseline: 1.3628x; 1.0310x over previous
"""Multi-head self-attention (B=2, S=4096, D=512, H=8, Dh=64) on 8 TRN2 cores.

Sharding: core i handles batch b = i//4 and head-pair hp = i%4 (heads 2*hp,
2*hp+1).  Each core computes Q/K/V projections for its two heads, flash-style
attention (no-max softmax; scores range is +-9 so exp is safe), and a partial
out-projection.  Host sums the 4 partial outputs per batch and transposes back.

v2 (bf16 pipeline): all matmul operands are bfloat16.  bf16 matmuls use
separate LDWEIGHTS instructions that the PE's 64-deep reorder window pulls
into the background weight buffer (plus FWL), so weight loads hide under the
streaming passes -- unlike fp32r matmuls, which self-load weights serially
(~107ns per matmul).  The kernel is restructured so the ACT engine (exp,
1 elem/cycle/lane @1.2GHz = the softmax roofline, ~290us for 33.5M exps)
never stalls:
  - scores PSUM tiles double-buffered, ctx PSUM double-buffered (normalize of
    block i runs under block i+1's compute; no PE idle -> no HAM re-throttle)
  - K projection accumulates chunk-by-chunk as the X DMA lands
  - out-projection of q-block i runs on the spare ctx-ring slot inside block
    i+1, copies on DVE (never ACT), output DMA overlapped
  - softmax normalize via reciprocal_approx_fast + DRAM-bounce partition
    broadcast, all off the critical path

Layouts (feature dim on partitions; every matmul contracts on partitions):
  xt  [512, S]  = X[b].T                       (bf16)
  wq/wk/wv [512, 128] = W[:, hp*128:(hp+1)*128] (bf16)
  wo  [128, 512] = Wo[hp*128:(hp+1)*128, :]     (bf16)
  yt  [512, S]  = partial (Y[b]).T              (fp32)

TRN2 quirk: walrus legalizes only ONE sync wait on TPB compute instructions.
`_legalize_matmul_waits` moves extra waits onto injected single-wait no-ops.
"""

import sys
from contextlib import ExitStack

for _p in ("/opt/trn_rl_repo",):
    if _p not in sys.path:
        sys.path.insert(0, _p)

import numpy as np

import concourse.bass as bass
import concourse.tile as tile
from concourse import mybir
from concourse.bass_utils import run_bass_kernel_spmd
from concourse.masks import make_identity

F32 = mybir.dt.float32
BF16 = mybir.dt.bfloat16
MM_DT = BF16
D = 512          # model dim
DH = 64          # head dim
P = 128          # partitions
B = 2
H = 8
S_FULL = 4096
N_CORES = 8
NC_T = D // P    # 4 contraction tiles over model dim

LAST_RESULTS = None  # test harness reads exec_time_ns from here


def _emit(nc: bass.Bass, tc: "tile.TileContext", ctx: ExitStack, S: int):
    """Emit the per-core program. Parameterized by S for small-sim testing."""
    NK = S // P              # 128-row key tiles
    QB = 512                 # q-block (both heads processed per block)
    NQB = S // QB            # attention q-blocks
    inv_scale = 1.0 / np.sqrt(DH)

    def mm(out, lhsT, rhs, start=True, stop=True):
        return nc.tensor.matmul(out, lhsT, rhs, start=start, stop=stop)

    xt = nc.declare_dram_parameter("xt", [D, S], MM_DT, isOutput=False)
    wq = nc.declare_dram_parameter("wq", [D, P], MM_DT, isOutput=False)
    wk = nc.declare_dram_parameter("wk", [D, P], MM_DT, isOutput=False)
    wv = nc.declare_dram_parameter("wv", [D, P], MM_DT, isOutput=False)
    wo = nc.declare_dram_parameter("wo", [P, D], MM_DT, isOutput=False)
    yt = nc.declare_dram_parameter("yt", [D, S], F32, isOutput=True)

    const = ctx.enter_context(tc.tile_pool(name="const", bufs=1))

    # ---- weight DMA first (ONE dma each: a dma_start costs ~650ns of Sync
    # issue time regardless of size), then X in 8 pieces, first half first,
    # so the K projection for q-block 0 can start early ----
    w_sb = {}
    for name, ap in (("wk", wk), ("wq", wq), ("wv", wv)):
        w4 = const.tile([P, NC_T * P], MM_DT, tag=f"{name}4", name=f"{name}4")
        wap = ap[:, :]
        base = wap.ap          # [[row_stride, 512], [col_stride, 128]]
        src = bass.AP(tensor=wap.tensor, offset=wap.offset,
                      ap=[[base[0][0], P], [base[0][0] * P, NC_T],
                          [base[1][0], P]])
        nc.sync.dma_start(out=w4.rearrange("p (c j) -> p c j", c=NC_T), in_=src)
        w_sb[name] = [w4[:, c * P:(c + 1) * P] for c in range(NC_T)]
    wo_sb = const.tile([P, D], MM_DT, tag="wo")
    nc.sync.dma_start(out=wo_sb[:], in_=wo[:, :])
    xt_sb = [const.tile([P, S], MM_DT, tag=f"xt{c}", name=f"xt{c}")
             for c in range(NC_T)]
    XPIECE = min(2048, S)
    for b in range(S // XPIECE):
        for c in range(NC_T):
            sl = slice(b * XPIECE, (b + 1) * XPIECE)
            nc.sync.dma_start(out=xt_sb[c][:, sl], in_=xt[c * P:(c + 1) * P, sl])

    # persistent intermediates
    qt_sb = const.tile([P, S], MM_DT, tag="qt")      # [2*64 d, S] stacked heads
    kt_sb = const.tile([P, S], MM_DT, tag="kt")
    # V with a ones column appended per k-tile: [128 k, NK*65]; col 64 == 1.0
    vones = [const.tile([P, NK * (DH + 1)], MM_DT, tag=f"vones{h}", name=f"vones{h}")
             for h in range(2)]
    konst = const.tile([P, NK, 1], F32, tag="konst")
    nc.vector.memset(konst[:], 1.0)
    for h in range(2):
        vv = vones[h].rearrange("p (k c) -> p k c", c=DH + 1)
        nc.vector.tensor_copy(vv[:, :, DH:DH + 1], konst[:])
    ctx_sb = const.tile([P, S], MM_DT, tag="ctx")    # context^T, stacked heads

    # PSUM banks: "s" 2 x [128,1024] (4) + "ctx" 2 x [65,512] (2) + "pp" 2 x
    # [128,512] (2) = 8.  Buffer *addresses* are assigned by autobufs
    # (interval packing); bufs= here are the per-tag caps.
    ps = ctx.enter_context(tc.tile_pool(name="ps", bufs=2, space="PSUM"))
    es = ctx.enter_context(tc.tile_pool(name="es", bufs=8))
    bcp = ctx.enter_context(tc.tile_pool(name="bcp", bufs=2))
    rtp = ctx.enter_context(tc.tile_pool(name="rtp", bufs=2))
    rdp = ctx.enter_context(tc.tile_pool(name="rdp", bufs=2, space="DRAM"))
    osb = ctx.enter_context(tc.tile_pool(name="osb", bufs=4))
    vtp = ctx.enter_context(tc.tile_pool(name="vtp", bufs=2))

    ident = const.tile([P, P], MM_DT, tag="ident")
    make_identity(nc, ident[:])

    # ---- projections (tag "pp"; they fill PE slack under the ACT-bound
    # attention loop, racing ahead of their consumers) ----
    def proj_block(dst, wname, lo):
        """dst[:, lo:lo+512] = (W^T x)[:, lo:lo+512] over the 4 chunks."""
        pq = ps.tile([P, 512], F32, tag="pp", bufs=2, name=f"p{wname}")
        for c in range(NC_T):
            mm(pq[:], w_sb[wname][c][:], xt_sb[c][:, lo:lo + 512],
               start=(c == 0), stop=(c == NC_T - 1))
        nc.vector.tensor_copy(dst[:, lo:lo + 512], pq[:])

    def proj_v_group(g):
        """V for k-tiles 4g..4g+3, v-major N=512 matmuls (one stationary per
        chunk instead of per k-tile -- V at N=128 is LDWEIGHTS-bound), then
        PE-transposed back to k-major for the vones layout."""
        sl = slice(g * 512, (g + 1) * 512)
        pv = ps.tile([P, 512], F32, tag="pp", bufs=2, name="pv")
        for c in range(NC_T):
            mm(pv[:], w_sb["wv"][c][:], xt_sb[c][:, sl],
               start=(c == 0), stop=(c == NC_T - 1))
        vtmp = vtp.tile([P, 512], MM_DT, tag="vt", name="vtmp")
        nc.vector.tensor_copy(vtmp[:], pv[:])
        pt = ps.tile([P, 512], MM_DT, tag="pp", bufs=2, name="pt")
        for j in range(4):
            nc.tensor.transpose(pt[:, j * P:(j + 1) * P],
                                vtmp[:, j * P:(j + 1) * P], ident[:])
        for j in range(4):
            k = 4 * g + j
            for h in range(2):
                nc.vector.tensor_copy(
                    vones[h][:, k * (DH + 1):k * (DH + 1) + DH],
                    pt[:, j * P + h * DH:j * P + (h + 1) * DH])

    # upfront on the critical path to the first exp: only K and Q for
    # q-block 0 (the first scores need nothing else); everything later
    # races the attention loop (Q blocks are emitted inside the loop, 4
    # k-iterations before their consumer)
    NSB = S // 512
    proj_block(kt_sb, "wk", 0)
    proj_block(qt_sb, "wq", 0)

    # ---- phase B + C interleaved ----
    # Per (qb, k): the two heads' score matmuls are row-packed -- h0 uses PE
    # rows 0-63 (base_partition 0), h1 rows 64-127 (base_partition 64) -- and
    # run CONCURRENTLY in disjoint row-groups, writing the two 512-col halves
    # (= two different banks) of one [128,1024] PSUM tile.  A single N=1024
    # exp then covers both heads, keeping the ACT cadence at (1024+352)/1.2
    # ~= 1147ns per k-tile while PE streaming is only ~650ns (fits under the
    # ACT cadence even when the power manager halves the PE clock).
    def normalize(h, qb, ctx_ps):
        """Rowsum reciprocal + partition-broadcast for one head-block.
        The [1,QB] rowsum row is reshaped to [64, QB//64] via a DRAM bounce
        so the DVE reciprocal costs ~QB//64 columns, not QB.  Returns the
        [128, QB] broadcast tile of 1/rowsum (consumed ONLY by the DVE
        combine of the out-projection -- never gates the PE)."""
        NW = QB // DH        # columns per partition in the [64, NW] reshape
        rt = rtp.tile([1, QB], F32, tag="rt", name="rt")
        nc.vector.tensor_copy(rt[0:1, :], ctx_ps[DH:DH + 1, :])
        rd1 = rdp.tile([1, QB], F32, tag="rd1", name="rd1")
        nc.sync.dma_start(out=rd1[:], in_=rt[0:1, :])
        el = list(rd1[0:1, :].ap)[1]           # [elem_stride, QB]
        rd1_64 = bass.AP(tensor=rd1.tensor, offset=rd1.offset,
                         ap=[[el[0] * NW, DH], [el[0], NW]])
        r64 = rtp.tile([DH, NW], F32, tag="r64", name="r64")
        nc.sync.dma_start(out=r64[:], in_=rd1_64)
        rinv64 = rtp.tile([DH, NW], F32, tag="rinv64", name="rinv64")
        nc.vector.reciprocal(rinv64[:], r64[:])
        rd2 = rdp.tile([1, QB], F32, tag="rd2", name="rd2")
        el2 = list(rd2[0:1, :].ap)[1]
        rd2_64 = bass.AP(tensor=rd2.tensor, offset=rd2.offset,
                         ap=[[el2[0] * NW, DH], [el2[0], NW]])
        nc.sync.dma_start(out=rd2_64, in_=rinv64[:])
        rd2_bcast = bass.AP(tensor=rd2.tensor, offset=rd2.offset,
                            ap=[[0, P], el2])
        bc = bcp.tile([P, QB], F32, tag="bc", name="bc")
        nc.sync.dma_start(out=bc[:], in_=rd2_bcast)
        return bc

    def out_mm(prev, idx, tag="pp"):
        """Out-projection tile idx for the PREVIOUS q-block: two K=64
        row-packed matmuls against the UNNORMALIZED context (ready right at
        the block boundary), staged out of PSUM immediately.  The 1/rowsum
        scales are applied later by out_combine on the DVE."""
        qsl, raws = prev["qsl"], []
        for h in range(2):
            hsl = slice(h * DH, (h + 1) * DH)
            o_ps = ps.tile([P, QB], F32, tag=tag, bufs=2, name="o_ps")
            mm(o_ps[:], wo_sb[hsl, idx * P:(idx + 1) * P], ctx_sb[hsl, qsl])
            o_raw = osb.tile([P, QB], F32, tag="oraw", bufs=4, name="o_raw")
            nc.vector.tensor_copy(o_raw[:], o_ps[:])
            raws.append(o_raw)
        prev["raw"][idx] = raws

    def out_combine(prev, idx):
        """o = o_h0 * bc_h0 + o_h1 * bc_h1, then DMA out.  One mul rides the
        (otherwise idle) GpSimd engine."""
        r0, r1 = prev["raw"][idx]
        m0 = osb.tile([P, QB], F32, tag="m0", bufs=2, name="m0")
        nc.vector.tensor_mul(m0[:], r0[:], prev["bc"][0][:])
        m1 = osb.tile([P, QB], F32, tag="m1", bufs=2, name="m1")
        nc.gpsimd.tensor_mul(m1[:], r1[:], prev["bc"][1][:])
        o_sb = osb.tile([P, QB], F32, tag="osb", bufs=2, name="o_sb")
        nc.vector.tensor_add(o_sb[:], m0[:], m1[:])
        nc.sync.dma_start(out=yt[idx * P:(idx + 1) * P, prev["qsl"]],
                          in_=o_sb[:])

    def emit_scores(qb, k):
        qsl = slice(qb * QB, (qb + 1) * QB)
        s_pair = ps.tile([P, 2 * QB], F32, tag="s", name="s_pair")
        for h in range(2):
            hsl = slice(h * DH, (h + 1) * DH)
            mm(s_pair[:, h * QB:(h + 1) * QB],
               kt_sb[hsl, k * P:(k + 1) * P], qt_sb[hsl, qsl])
        return s_pair

    # Software-pipelined main loop: iteration i+1's score matmuls are
    # emitted (= prioritized) ahead of iteration i's ctx matmuls, so at a
    # q-block boundary the next block's first scores run before the old
    # block's final ctx pair and the exp stream never waits.
    iters = [(qb, k) for qb in range(NQB) for k in range(NK)]
    ctx_blocks = {}
    prev = None
    spair_next = emit_scores(0, 0)
    # remaining projections, emitted after the first scores so they don't
    # delay the first exp; the scheduler runs them in PE slack.  K block g
    # and V group g are both consumed from score/ctx k-tile 4g on.
    if NSB > 1:
        proj_block(kt_sb, "wk", 512)
    proj_v_group(0)
    for g in range(1, NSB):
        if g >= 2:
            proj_block(kt_sb, "wk", g * 512)
        proj_v_group(g)
    for i, (qb, k) in enumerate(iters):
        qsl = slice(qb * QB, (qb + 1) * QB)
        if k == 0:
            ctx_blocks[qb] = [ps.tile([DH + 1, QB], F32, tag="ctx", bufs=2,
                                      name=f"ctx_ps{h}") for h in range(2)]
        s_pair, ctx_h = spair_next, ctx_blocks[qb]
        if i + 1 < len(iters):
            spair_next = emit_scores(*iters[i + 1])
        e_pair = es.tile([P, 2 * QB], MM_DT, tag="e", name="e_pair")
        nc.scalar.activation(e_pair[:], s_pair[:],
                             mybir.ActivationFunctionType.Exp,
                             scale=inv_scale)
        for h in range(2):
            vo = vones[h][:, k * (DH + 1):(k + 1) * (DH + 1)]
            mm(ctx_h[h][:], vo, e_pair[:, h * QB:(h + 1) * QB],
               start=(k == 0), stop=(k == NK - 1))
        # previous q-block's out-projection, spread over this block's
        # PE/DVE slack: matmuls (vs unnormalized ctx -- no wait on the
        # normalize chain) at k=0..3, scale-combines at k=4..7
        if prev is not None:
            if k < NC_T:
                out_mm(prev, k)
            elif k < 2 * NC_T:
                out_combine(prev, k - NC_T)
        # next q-block's Q projection, 4 iterations before its consumer
        # (the score-ahead emission at k == NK-1)
        if k == NK - 4 and qb + 1 < NQB:
            proj_block(qt_sb, "wq", (qb + 1) * 512)
        if k == NK - 1:
            last = (qb == NQB - 1)
            if not last:
                # stage the unnormalized context out of PSUM (bf16); the
                # 1/rowsum scales are applied by out_combine later
                for h in range(2):
                    nc.vector.tensor_copy(ctx_sb[h * DH:(h + 1) * DH, qsl],
                                          ctx_h[h][:DH, :])
            prev = {"qsl": qsl, "raw": [None] * NC_T,
                    "bc": [normalize(h, qb, ctx_blocks[qb][h])
                           for h in range(2)]}
    # final q-block: nothing left to pipeline against, so take the
    # short path -- scale the context in place, then full-K out-projection
    qsl = prev["qsl"]
    for h in range(2):
        nc.vector.tensor_mul(ctx_sb[h * DH:(h + 1) * DH, qsl],
                             ctx_blocks[NQB - 1][h][:DH, :],
                             prev["bc"][h][0:DH, :])
    for idx in range(NC_T):
        o_ps = ps.tile([P, QB], F32, tag="s", name="o_ps")
        mm(o_ps[:], wo_sb[:, idx * P:(idx + 1) * P], ctx_sb[:, qsl])
        o_sb = osb.tile([P, QB], F32, tag="osb", bufs=2, name="o_sb")
        nc.vector.tensor_copy(o_sb[:], o_ps[:])
        nc.sync.dma_start(out=yt[idx * P:(idx + 1) * P, qsl], in_=o_sb[:])


_TPB_ENGINES = {mybir.EngineType.PE, mybir.EngineType.Activation,
                mybir.EngineType.DVE, mybir.EngineType.Pool}


def _legalize_matmul_waits(nc: bass.Bass) -> int:
    """Walrus encodes only ONE sync wait on TPB compute instructions (seen on
    Matmult and TensorCopy).  Move extra waits onto injected same-engine
    no-ops (one wait each) placed immediately before the instruction in its
    block: same semantics, legal encoding."""
    n_fixed = 0
    for f in nc.m.functions:
        for bb in f.blocks:
            out = []
            changed = False
            for ins in bb.instructions:
                si = ins.sync_info
                if (getattr(ins, "engine", None) is not None
                        and si is not None and len(si.on_wait) > 1):
                    for idx, w in enumerate(si.on_wait[:-1]):
                        nop = mybir.InstNoOp(name=f"{ins.name}-lgw{idx}",
                                             ins=[], outs=[])
                        nop.engine = ins.engine
                        nop.sync_info = mybir.SyncInfo(on_wait=[w], on_update=[])
                        out.append(nop)
                    ins.sync_info = mybir.SyncInfo(on_wait=[si.on_wait[-1]],
                                                   on_update=si.on_update)
                    n_fixed += 1
                    changed = True
                out.append(ins)
            if changed:
                bb.instructions = out
    return n_fixed


def build(S: int = S_FULL, legalize: bool = False) -> bass.Bass:
    nc = bass.Bass()
    with ExitStack() as ctx:
        ctx.enter_context(nc.allow_low_precision(
            reason="bf16 matmul operands / intermediates"))
        tc = ctx.enter_context(tile.TileContext(nc))
        _emit(nc, tc, ctx, S)
    if legalize:
        # only for the walrus/hardware path; CoreSim wants updates on every
        # instruction and doesn't enforce the 1-wait Matmult limit
        _legalize_matmul_waits(nc)
    return nc


_NC_CACHE = {}


def _get_nc(S: int) -> bass.Bass:
    if S not in _NC_CACHE:
        _NC_CACHE[S] = build(S, legalize=True)
    return _NC_CACHE[S]


def _bf16(a):
    import ml_dtypes
    return np.ascontiguousarray(np.asarray(a, dtype=np.float32)).astype(
        ml_dtypes.bfloat16)


def make_in_maps(X, Wq, Wk, Wv, Wo):
    X = np.asarray(X, dtype=np.float32)
    xts = [_bf16(X[b].T) for b in range(B)]
    in_maps = []
    for i in range(N_CORES):
        b, hp = divmod(i, 4)  # 4 head-pairs per batch
        csl = slice(hp * P, (hp + 1) * P)
        in_maps.append({
            "xt": xts[b],
            "wq": _bf16(Wq[:, csl]),
            "wk": _bf16(Wk[:, csl]),
            "wv": _bf16(Wv[:, csl]),
            "wo": _bf16(Wo[csl, :]),
        })
    return in_maps


def kernel(X, Wq, Wk, Wv, Wo, _trace=False):
    global LAST_RESULTS
    X = np.asarray(X, dtype=np.float32)
    S = X.shape[1]
    nc = _get_nc(S)
    in_maps = make_in_maps(X, np.asarray(Wq, np.float32), np.asarray(Wk, np.float32),
                           np.asarray(Wv, np.float32), np.asarray(Wo, np.float32))
    res = run_bass_kernel_spmd(nc, in_maps, list(range(N_CORES)), trace=_trace)
    LAST_RESULTS = res
    Y = np.zeros((B, S, D), dtype=np.float32)
    for i in range(N_CORES):
        Y[i // 4] += res.results[i]["yt"].T
    return Y
